# revision 3
# baseline (speedup 1.0000x reference)
"""Bass/Trainium2 kernel for a ragged-sequence CrossAttentionBlock.

Math (per reference):
  T = 16*196 packed tokens, D=512, H=8 heads of HD=64.
  q = (xq + pos) @ Wq + bq ; k = (xk + pos) @ Wk + bk ; v = xk @ Wv + bv
  block-diagonal attention over segments of channels[i]*196 tokens
  out = softmax(q k^T / 8) v  -> concat heads -> @ Wo + bo

Sharding: one head per NeuronCore. Each core computes its head's Q/K/V
over all tokens, the per-segment attention, and its head's slice of the
output projection Wo[h*64:(h+1)*64, :]. Host sums per-head projections.

Bias folding (exact):
  bk: uniform per-query logit shift -> softmax-invariant, drops.
  bq: per-partition add on Q^T after projection (emitted only if nonzero).
  bv, bo: commute through softmax (rows sum to 1): host adds bv@Wo + bo.

v2 design notes (vs the padded baseline):
  - tight token axis: inputs/outputs are [128, 4, T] with NO inter-segment
    padding; projection blocks coincide with the 512-token query blocks of
    each segment, so K/V granules align to segment starts.
  - all input DMAs ride ONE queue (sync) in priority order, throttled by
    per-tag pool bufs, so the first block lands in ~4us instead of ~23us.
  - K^T tiles all live in partitions 0-63 (copied from the QK PSUM rows
    64-127 with a partition shift), so Q^T needs no duplicate.
  - softmax sums ride the AV PSUM as row 64 and are cast into row 64 of
    the o2 staging tensor (bf16) -- no separate sums copy.
  - a software pipeline at query-block granularity: while block j's scores
    stream and exp (the pacer, on the scalar engine) runs, the PE executes
    "filler" work -- AV of block j-1, outproj of block j-2, projections of
    the next segment -- popped from a deque sized by an ACT-vs-PE debt
    estimate. PSUM: proj(1) + scores(4) + av(1) + outproj(2) = 8 banks.
  - dtypes: everything bf16 except xkp (fp8e4m3, as the error budget
    allows exactly one fp8 quantization under the 2e-2 gate).
"""

import sys
import types
from collections import deque

import numpy as np
import ml_dtypes

_D = 512
_HD = 64
_H = 8
_S = 196
_NCORES = 8

_prog_cache = {}


def _ensure_ntff_hook():
    """Register the NTFF profile hook that the agent image's antenv lacks."""
    if "antenv.axon_hooks" in sys.modules:
        return
    try:
        from trn_agent_boot.trn_boot import _ntff_profile_via_ctypes

        hook = _ntff_profile_via_ctypes("/opt/axon/libaxon_pjrt.so")
        mod = types.ModuleType("antenv.axon_hooks")
        mod.get_axon_ntff_profile_hook = lambda: hook
        sys.modules["antenv.axon_hooks"] = mod
    except Exception:
        pass


def _segments(channels):
    seg_len = [int(c) * _S for c in np.asarray(channels).tolist() if int(c) > 0]
    toff = []
    o = 0
    for l in seg_len:
        toff.append(o)
        o += l
    return seg_len, toff, o  # lengths, tight offsets, T


def _plan(seg_len, toff, T):
    nseg = len(seg_len)
    nkt_s = [(l + 127) // 128 for l in seg_len]
    slot_base = []
    b = 0
    for s in range(nseg):
        slot_base.append(b)
        b += nkt_s[s]
    NK = b
    p_order = sorted(range(nseg), key=lambda s: (seg_len[s], s))
    blocks = []  # (s, qb0, W, goff)
    for s in p_order:
        for qb0 in range(0, seg_len[s], 512):
            W = min(512, seg_len[s] - qb0)
            blocks.append((s, qb0, W, toff[s] + qb0))
    return dict(nseg=nseg, nkt_s=nkt_s, slot_base=slot_base, NK=NK,
                p_order=p_order, blocks=blocks, NQB=len(blocks),
                max_nkt=max(nkt_s))


def _build_program(seg_key, bq_nonzero):
    import concourse.bacc as bacc
    import concourse.tile as tile
    from concourse import mybir

    seg_len, toff, T = seg_key
    seg_len, toff = list(seg_len), list(toff)
    P = _plan(seg_len, toff, T)
    nseg, nkt_s, slot_base = P["nseg"], P["nkt_s"], P["slot_base"]
    NK, blocks, NQB, max_nkt = P["NK"], P["blocks"], P["NQB"], P["max_nkt"]

    f32 = mybir.dt.float32
    bf16 = mybir.dt.bfloat16
    f8e4 = mybir.dt.float8e4

    nc = bacc.Bacc("TRN2", target_bir_lowering=False, debug=False,
                   num_devices=_NCORES)

    xqp_d = nc.dram_tensor("xqp", [128, 4, T], bf16, kind="ExternalInput")
    xkp_d = nc.dram_tensor("xkp", [128, 4, T], f8e4, kind="ExternalInput")
    xk_d = nc.dram_tensor("xk", [128, 4, T], bf16, kind="ExternalInput")
    wq_d = nc.dram_tensor("wq", [128, 4, _HD], bf16, kind="ExternalInput")
    wk_d = nc.dram_tensor("wk", [128, 4, _HD], bf16, kind="ExternalInput")
    wv_d = nc.dram_tensor("wv", [128, 4, _HD], bf16, kind="ExternalInput")
    wo_d = nc.dram_tensor("wo", [_HD, 4, 128], bf16, kind="ExternalInput")
    if bq_nonzero:
        bq_d = nc.dram_tensor("bq", [_HD, 1], f32, kind="ExternalInput")
    outT_d = nc.dram_tensor("outT", [128, 4, T], bf16, kind="ExternalOutput")
    sums_d = nc.dram_tensor("sums", [1, NQB, 512], bf16,
                            kind="ExternalOutput")

    with tile.TileContext(nc) as tc:
        with (
            tc.tile_pool(name="consts", bufs=1) as consts,
            tc.tile_pool(name="persist", bufs=1) as persist,
            tc.tile_pool(name="xin", bufs=2) as xin,
            tc.tile_pool(name="sbw", bufs=2) as sbw,
            tc.tile_pool(name="expp", bufs=3) as expp,
            tc.tile_pool(name="fins", bufs=2) as fins,
            tc.tile_pool(name="pjp", bufs=1, space="PSUM") as pjp,
            tc.tile_pool(name="scp", bufs=1, space="PSUM") as scp,
            tc.tile_pool(name="avp", bufs=1, space="PSUM") as avp,
            tc.tile_pool(name="fip", bufs=2, space="PSUM") as fip,
        ):
            wq_sb = consts.tile([128, 4, _HD], bf16)
            wk_sb = consts.tile([128, 4, _HD], bf16)
            wv_sb = consts.tile([128, 4, _HD], bf16)
            wo_sb = consts.tile([_HD, 4, 128], bf16)
            nc.sync.dma_start(out=wq_sb, in_=wq_d[:, :, :])
            nc.sync.dma_start(out=wk_sb, in_=wk_d[:, :, :])
            nc.sync.dma_start(out=wv_sb, in_=wv_d[:, :, :])
            nc.sync.dma_start(out=wo_sb, in_=wo_d[:, :, :])
            if bq_nonzero:
                bq_sb = consts.tile([_HD, 1], f32)
                nc.sync.dma_start(out=bq_sb, in_=bq_d[:, :])
            wdummy = consts.tile([128, 128], bf16)
            nc.vector.memset(wdummy, 0.125)

            # persistent per-head tensors
            q2_sb = persist.tile([_HD, T], bf16)         # Q^T (parts 0-63)
            k2_sb = persist.tile([_HD, NK, 128], bf16)   # K^T granules
            v_all = persist.tile([128, NK, _HD + 1], bf16)  # V rows + ones
            o2_sb = persist.tile([_HD + 1, NQB, 512], bf16)  # AV out + sums

            # granule-tail init: zero K^T tails, zero V tails, set ones col
            for s in range(nseg):
                g0 = slot_base[s]
                nfull = seg_len[s] // 128
                rem = seg_len[s] - nfull * 128
                if nfull:
                    nc.gpsimd.memset(v_all[:, g0:g0 + nfull, _HD:_HD + 1], 1.0)
                if rem:
                    g = g0 + nfull
                    nc.vector.memset(k2_sb[:, g, rem:128], 0.0)
                    nc.gpsimd.memset(v_all[:, g, :], 0.0)
                    nc.gpsimd.memset(v_all[0:rem, g, _HD:_HD + 1], 1.0)

            # -------- input DMAs: one queue, priority order, throttled ----
            xq_tiles, xkp_tiles, xk_tiles = {}, {}, {}
            for bi, (s, qb0, W, goff) in enumerate(blocks):
                xq_t = xin.tile([128, 4, 512], bf16, tag="xq", name="xq",
                                bufs=2)
                xkp_t = xin.tile([128, 4, 512], f8e4, tag="xkp", name="xkp",
                                 bufs=2)
                xk_t = xin.tile([128, 4, 512], bf16, tag="xk", name="xk",
                                bufs=3)
                nc.sync.dma_start(out=xq_t[:, :, 0:W],
                                  in_=xqp_d[:, :, goff:goff + W])
                nc.sync.dma_start(out=xkp_t[:, :, 0:W],
                                  in_=xkp_d[:, :, goff:goff + W])
                nc.sync.dma_start(out=xk_t[:, :, 0:W],
                                  in_=xk_d[:, :, goff:goff + W])
                xq_tiles[bi], xkp_tiles[bi], xk_tiles[bi] = xq_t, xkp_t, xk_t

            # -------- warmup: lift the PE clock gate during the DMA head --
            warm = scp.tile([128, 4, 512], f32, tag="sc", name="warm")
            for _ in range(30):
                nc.tensor.matmul(warm[:, 0, 0:128], lhsT=wdummy, rhs=wdummy,
                                 start=True, stop=True)

            # -------- emission helpers ------------------------------------
            pj_ps = {}

            def qk_mm(bi):
                s, qb0, W, goff = blocks[bi]
                ps = pjp.tile([128, 512], f32, tag="pj", name="qkps")
                for c in range(4):
                    nc.tensor.matmul(ps[0:_HD, 0:W], lhsT=wq_sb[:, c],
                                     rhs=xq_tiles[bi][:, c, 0:W],
                                     start=(c == 0), stop=(c == 3))
                    nc.tensor.matmul(ps[_HD:128, 0:W], lhsT=wk_sb[:, c],
                                     rhs=xkp_tiles[bi][:, c, 0:W],
                                     start=(c == 0), stop=(c == 3))
                pj_ps[bi] = ps

            def qk_cast(bi):
                s, qb0, W, goff = blocks[bi]
                ps = pj_ps.pop(bi)
                nc.vector.tensor_copy(out=q2_sb[:, goff:goff + W],
                                      in_=ps[0:_HD, 0:W])
                if bq_nonzero:
                    nc.vector.tensor_scalar_add(
                        q2_sb[:, goff:goff + W], q2_sb[:, goff:goff + W],
                        bq_sb[:, 0:1])
                l0 = slot_base[s] + qb0 // 128
                nbf, rem = W // 128, W % 128
                if nbf:
                    nc.vector.tensor_copy(
                        out=k2_sb[:, l0:l0 + nbf, :],
                        in_=ps[_HD:128, 0:nbf * 128].rearrange(
                            "p (b t) -> p b t", b=nbf, t=128))
                if rem:
                    nc.vector.tensor_copy(
                        out=k2_sb[:, l0 + nbf, 0:rem],
                        in_=ps[_HD:128, nbf * 128:W])

            def v_mm(bi):
                s, qb0, W, goff = blocks[bi]
                ps = pjp.tile([128, 512], f32, tag="pj", name="vtps")
                for c in range(4):
                    nc.tensor.matmul(ps[0:_HD, 0:W], lhsT=wv_sb[:, c],
                                     rhs=xk_tiles[bi][:, c, 0:W],
                                     start=(c == 0), stop=(c == 3))
                pj_ps[bi] = ps

            def v_tail(bi):
                s, qb0, W, goff = blocks[bi]
                ps = pj_ps.pop(bi)
                nb = (W + 127) // 128
                nbf, rem = W // 128, W % 128
                vt = sbw.tile([_HD, 512], bf16, tag="vt", name="vt")
                nc.vector.tensor_copy(out=vt[:, 0:nb * 128],
                                      in_=ps[0:_HD, 0:nb * 128])
                stag = sbw.tile([128, 4, _HD], bf16, tag="stag", name="stag")
                nc.scalar.dma_start_transpose(stag[:, 0:nb, :],
                                              vt[:, 0:nb * 128])
                l0 = slot_base[s] + qb0 // 128
                if nbf:
                    nc.gpsimd.tensor_copy(
                        out=v_all[:, l0:l0 + nbf, 0:_HD],
                        in_=stag[:, 0:nbf, :])
                if rem:
                    nc.gpsimd.tensor_copy(
                        out=v_all[0:rem, l0 + nbf, 0:_HD],
                        in_=stag[0:rem, nbf, :])

            # -------- filler machinery ------------------------------------
            fillers = deque()  # (cost_ns, fn)
            debt = [0.0]

            def pop_fillers():
                while debt[0] > 0 and fillers:
                    cost, fn = fillers.popleft()
                    fn()
                    debt[0] -= cost

            def drain_fillers():
                while fillers:
                    _, fn = fillers.popleft()
                    fn()
                debt[0] = 0.0

            def enq_qk(s):
                for bi, (s2, qb0, W, goff) in enumerate(blocks):
                    if s2 != s:
                        continue
                    fillers.append((W * 1.7, lambda b=bi: qk_mm(b)))
                    fillers.append((0.0, lambda b=bi: qk_cast(b)))

            def enq_v(s):
                for bi, (s2, qb0, W, goff) in enumerate(blocks):
                    if s2 != s:
                        continue
                    fillers.append((W * 0.85, lambda b=bi: v_mm(b)))
                    fillers.append((0.0, lambda b=bi: v_tail(b)))

            ex_tiles = {}

            def emit_scores(j):
                s, qb0, W, goff = blocks[j]
                nkt = nkt_s[s]
                sb = slot_base[s]
                ex = expp.tile([128, max_nkt, 512], bf16, tag="ex", name="ex")
                ex_tiles[j] = ex
                lk = 0
                while lk < nkt:
                    G = min(4, nkt - lk)
                    sc = scp.tile([128, 4, 512], f32, tag="sc", name="sc")
                    for g in range(G):
                        nc.tensor.matmul(
                            sc[:, g, 0:W], lhsT=k2_sb[:, sb + lk + g, :],
                            rhs=q2_sb[:, goff:goff + W],
                            start=True, stop=True)
                    nc.scalar.activation(
                        out=ex[:, lk:lk + G, 0:W], in_=sc[:, 0:G, 0:W],
                        func=mybir.ActivationFunctionType.Exp, scale=0.125)
                    lk += G
                    debt[0] += G * W * 0.66
                    pop_fillers()

            def enq_av(j):
                s, qb0, W, goff = blocks[j]
                nkt = nkt_s[s]
                sb = slot_base[s]
                st = {}

                def av_l(l):
                    if l == 0:
                        st["o"] = avp.tile([_HD + 1, 512], f32, tag="av",
                                           name="ops")
                    nc.tensor.matmul(
                        st["o"][:, 0:W], lhsT=v_all[:, sb + l, :],
                        rhs=ex_tiles[j][:, l, 0:W],
                        start=(l == 0), stop=(l == nkt - 1))

                def o2cast():
                    nc.vector.tensor_copy(out=o2_sb[:, j, 0:W],
                                          in_=st["o"][:, 0:W])
                    ex_tiles.pop(j)

                for l in range(nkt):
                    fillers.append((W * 0.43, lambda l=l: av_l(l)))
                fillers.append((0.0, o2cast))

            def enq_outproj(j):
                s, qb0, W, goff = blocks[j]
                st = {}

                def op_c(c):
                    if c == 0:
                        st["f"] = fins.tile([128, 4, 512], bf16, tag="fin",
                                            name="fin")
                    f_ps = fip.tile([128, 512], f32, tag="fi", name="fps")
                    nc.tensor.matmul(f_ps[:, 0:W], lhsT=wo_sb[:, c, :],
                                     rhs=o2_sb[0:_HD, j, 0:W],
                                     start=True, stop=True)
                    nc.vector.tensor_copy(out=st["f"][:, c, 0:W],
                                          in_=f_ps[:, 0:W])

                def out_dma():
                    nc.gpsimd.dma_start(out=outT_d[:, :, goff:goff + W],
                                        in_=st["f"][:, :, 0:W])

                for c in range(4):
                    fillers.append((W * 0.43, lambda c=c: op_c(c)))
                fillers.append((0.0, out_dma))

            # -------- main pipeline ---------------------------------------
            first_seg = blocks[0][0]
            for bi, (s2, qb0, W, goff) in enumerate(blocks):
                if s2 == first_seg:
                    qk_mm(bi)
                    qk_cast(bi)

            seg_of = [b[0] for b in blocks]
            for j, (s, qb0, W, goff) in enumerate(blocks):
                if qb0 == 0 and j > 0:
                    drain_fillers()
                emit_scores(j)
                if qb0 == 0:
                    enq_v(s)
                    nxt = None
                    for jj in range(j + 1, NQB):
                        if seg_of[jj] != s:
                            nxt = seg_of[jj]
                            break
                    if nxt is not None:
                        enq_qk(nxt)
                enq_av(j)
                if j > 0:
                    enq_outproj(j - 1)
            drain_fillers()
            enq_outproj(NQB - 1)
            drain_fillers()
            nc.gpsimd.dma_start(out=sums_d[:, :, :],
                                in_=o2_sb[_HD:_HD + 1, :, :])

    nc.compile()
    return nc


def _prep_T(x, T, dtype):
    """[T, D] f32 -> [128, 4, T]: element (p, c, t) = x[t, c*128+p]."""
    xt = x.T.reshape(4, 128, T)  # [c, p, t]
    return np.ascontiguousarray(xt.transpose(1, 0, 2)).astype(dtype)


def kernel(x_query, x_keyval, pos, channels, Wq, bq, Wk, bk, Wv, bv, Wo, bo,
           _trace=False, _trace_cores=None):
    _ensure_ntff_hook()
    import concourse.bass_utils as bu

    bu.upload_artifacts = lambda tmpdir: tmpdir  # no S3 egress from here

    x_query = np.asarray(x_query, dtype=np.float32)
    x_keyval = np.asarray(x_keyval, dtype=np.float32)
    pos = np.asarray(pos, dtype=np.float32)
    channels = np.asarray(channels)
    Wq, bq = np.asarray(Wq, np.float32), np.asarray(bq, np.float32)
    Wk, bk = np.asarray(Wk, np.float32), np.asarray(bk, np.float32)
    Wv, bv = np.asarray(Wv, np.float32), np.asarray(bv, np.float32)
    Wo, bo = np.asarray(Wo, np.float32), np.asarray(bo, np.float32)

    C, S, D = x_query.shape
    seg_len, toff, T = _segments(channels)
    assert sum(seg_len) == C * S, "channels inconsistent with batch dim"
    P = _plan(seg_len, toff, T)

    bq_nonzero = bool(np.any(bq))
    seg_key = (tuple(seg_len), tuple(toff), T)
    cache_key = (seg_key, bq_nonzero)
    if cache_key not in _prog_cache:
        _prog_cache[cache_key] = _build_program(seg_key, bq_nonzero)
    nc = _prog_cache[cache_key]

    bf = ml_dtypes.bfloat16
    f8 = ml_dtypes.float8_e4m3fn
    xq_flat = x_query.reshape(-1, D)
    xk_flat = x_keyval.reshape(-1, D)
    p_flat = pos.reshape(-1, D)
    xqp_b = _prep_T(xq_flat + p_flat, T, bf)
    xkp_b = _prep_T(xk_flat + p_flat, T, f8)
    xk_b = _prep_T(xk_flat, T, bf)

    def wchunk(w):  # [512, 64] -> [128, 4, 64]
        return np.ascontiguousarray(
            w.reshape(4, 128, _HD).transpose(1, 0, 2)).astype(bf)

    in_maps = []
    for h in range(_NCORES):
        sl = slice(h * _HD, (h + 1) * _HD)
        m = {
            "xqp": xqp_b,
            "xkp": xkp_b,
            "xk": xk_b,
            "wq": wchunk(Wq[:, sl]),
            "wk": wchunk(Wk[:, sl]),
            "wv": wchunk(Wv[:, sl]),
            "wo": np.ascontiguousarray(
                Wo[sl, :].reshape(_HD, 4, 128)).astype(bf),
        }
        if bq_nonzero:
            m["bq"] = np.ascontiguousarray(
                bq[sl].reshape(_HD, 1)).astype(np.float32)
        in_maps.append(m)

    from concourse.bass_utils import run_bass_kernel_spmd

    kwargs = {}
    if _trace:
        kwargs["trace"] = True
        if _trace_cores is not None:
            kwargs["trace_cores"] = _trace_cores
    res = run_bass_kernel_spmd(nc, in_maps, list(range(_NCORES)), **kwargs)

    # host gather: per-head softmax normalization + sum + bias folds
    blocks = P["blocks"]
    acc = np.zeros((D, T), dtype=np.float64)
    with np.errstate(divide="ignore", invalid="ignore"):
        for h in range(_NCORES):
            outT = np.asarray(res.results[h]["outT"], dtype=np.float64)
            sums_r = np.asarray(res.results[h]["sums"], dtype=np.float64)
            sums = np.empty(T, dtype=np.float64)
            for j, (s, qb0, W, goff) in enumerate(blocks):
                sums[goff:goff + W] = sums_r[0, j, 0:W]
            sums = np.where(sums == 0.0, 1.0, sums)
            acc += outT.transpose(1, 0, 2).reshape(D, T) / sums[None, :]

    const = bv @ Wo + bo  # bias fold (exact; zero in the spec's fills)

    out = (acc.T + const[None, :]).astype(np.float32).reshape(C, S, D)

    if _trace:
        kernel._last_exec_time_ns = res.exec_time_ns
        kernel._last_trace = (
            res.instructions_and_trace[1] if res.instructions_and_trace else None
        )
    return out


# revision 10
# speedup vs baseline: 1.0970x; 1.0970x over previous
"""Bass/Trainium2 kernel for a ragged-sequence CrossAttentionBlock.

Math (per reference):
  T = 16*196 packed tokens, D=512, H=8 heads of HD=64.
  q = (xq + pos) @ Wq + bq ; k = (xk + pos) @ Wk + bk ; v = xk @ Wv + bv
  block-diagonal attention over segments of channels[i]*196 tokens
  out = softmax(q k^T / 8) v  -> concat heads -> @ Wo + bo

Sharding: one head per NeuronCore. Each core computes its head's Q/K/V
over all tokens, the per-segment attention, and its head's slice of the
output projection Wo[h*64:(h+1)*64, :]. Host sums per-head projections.

Bias folding (exact):
  bk: uniform per-query logit shift -> softmax-invariant, drops.
  bq: per-partition add on Q^T after projection (emitted only if nonzero).
  bv, bo: commute through softmax (rows sum to 1): host adds bv@Wo + bo.

v2 design notes (vs the padded baseline):
  - tight token axis: inputs/outputs are [128, 4, T] with NO inter-segment
    padding; projection blocks coincide with the 512-token query blocks of
    each segment, so K/V granules align to segment starts.
  - all input DMAs ride ONE queue (sync) in priority order, throttled by
    per-tag pool bufs, so the first block lands in ~4us instead of ~23us.
  - K^T tiles all live in partitions 0-63 (copied from the QK PSUM rows
    64-127 with a partition shift), so Q^T needs no duplicate.
  - softmax sums ride the AV PSUM as row 64 and are cast into row 64 of
    the o2 staging tensor (bf16) -- no separate sums copy.
  - a software pipeline at query-block granularity: while block j's scores
    stream and exp (the pacer, on the scalar engine) runs, the PE executes
    "filler" work -- AV of block j-1, outproj of block j-2, projections of
    the next segment -- popped from a deque sized by an ACT-vs-PE debt
    estimate. PSUM: proj(1) + scores(4) + av(1) + outproj(2) = 8 banks.
  - dtypes: everything bf16 except xkp (fp8e4m3, as the error budget
    allows exactly one fp8 quantization under the 2e-2 gate).
"""

import sys
import types
from collections import deque

import numpy as np
import ml_dtypes

_D = 512
_HD = 64
_H = 8
_S = 196
_NCORES = 8

_prog_cache = {}


def _ensure_ntff_hook():
    """Register the NTFF profile hook that the agent image's antenv lacks."""
    if "antenv.axon_hooks" in sys.modules:
        return
    try:
        from trn_agent_boot.trn_boot import _ntff_profile_via_ctypes

        hook = _ntff_profile_via_ctypes("/opt/axon/libaxon_pjrt.so")
        mod = types.ModuleType("antenv.axon_hooks")
        mod.get_axon_ntff_profile_hook = lambda: hook
        sys.modules["antenv.axon_hooks"] = mod
    except Exception:
        pass


def _segments(channels):
    seg_len = [int(c) * _S for c in np.asarray(channels).tolist() if int(c) > 0]
    toff = []
    o = 0
    for l in seg_len:
        toff.append(o)
        o += l
    return seg_len, toff, o  # lengths, tight offsets, T


def _plan(seg_len, toff, T):
    nseg = len(seg_len)
    nkt_s = [(l + 127) // 128 for l in seg_len]
    slot_base = []
    b = 0
    for s in range(nseg):
        slot_base.append(b)
        b += nkt_s[s]
    NK = b
    p_order = sorted(range(nseg), key=lambda s: (seg_len[s], s))
    blocks = []  # (s, qb0, W, goff)
    for s in p_order:
        for qb0 in range(0, seg_len[s], 512):
            W = min(512, seg_len[s] - qb0)
            blocks.append((s, qb0, W, toff[s] + qb0))
    return dict(nseg=nseg, nkt_s=nkt_s, slot_base=slot_base, NK=NK,
                p_order=p_order, blocks=blocks, NQB=len(blocks),
                max_nkt=max(nkt_s))


def _build_program(seg_key, bq_nonzero):
    import concourse.bacc as bacc
    import concourse.tile as tile
    from concourse import mybir

    seg_len, toff, T = seg_key
    seg_len, toff = list(seg_len), list(toff)
    P = _plan(seg_len, toff, T)
    nseg, nkt_s, slot_base = P["nseg"], P["nkt_s"], P["slot_base"]
    NK, blocks, NQB, max_nkt = P["NK"], P["blocks"], P["NQB"], P["max_nkt"]

    f32 = mybir.dt.float32
    bf16 = mybir.dt.bfloat16
    f8e4 = mybir.dt.float8e4

    nc = bacc.Bacc("TRN2", target_bir_lowering=False, debug=False,
                   num_devices=_NCORES)

    xqp_d = nc.dram_tensor("xqp", [128, 4, T], bf16, kind="ExternalInput")
    xkp_d = nc.dram_tensor("xkp", [128, 4, T], f8e4, kind="ExternalInput")
    xk_d = nc.dram_tensor("xk", [128, 4, T], bf16, kind="ExternalInput")
    wq_d = nc.dram_tensor("wq", [128, 4, _HD], bf16, kind="ExternalInput")
    wk_d = nc.dram_tensor("wk", [128, 4, _HD], bf16, kind="ExternalInput")
    wv_d = nc.dram_tensor("wv", [128, 4, _HD], bf16, kind="ExternalInput")
    wo_d = nc.dram_tensor("wo", [_HD, 4, 128], bf16, kind="ExternalInput")
    if bq_nonzero:
        bq_d = nc.dram_tensor("bq", [_HD, 1], f32, kind="ExternalInput")
    outT_d = nc.dram_tensor("outT", [128, 4, T], bf16, kind="ExternalOutput")
    sums_d = nc.dram_tensor("sums", [1, NQB, 512], bf16,
                            kind="ExternalOutput")

    with tile.TileContext(nc) as tc:
        with (
            tc.tile_pool(name="consts", bufs=1) as consts,
            tc.tile_pool(name="persist", bufs=1) as persist,
            tc.tile_pool(name="xin", bufs=2) as xin,
            tc.tile_pool(name="sbw", bufs=2) as sbw,
            tc.tile_pool(name="expp", bufs=3) as expp,
            tc.tile_pool(name="fins", bufs=2) as fins,
            tc.tile_pool(name="pjp", bufs=1, space="PSUM") as pjp,
            tc.tile_pool(name="scp", bufs=1, space="PSUM") as scp,
            tc.tile_pool(name="avp", bufs=1, space="PSUM") as avp,
            tc.tile_pool(name="fip", bufs=2, space="PSUM") as fip,
        ):
            wq_sb = consts.tile([128, 4, _HD], bf16)
            wk_sb = consts.tile([128, 4, _HD], bf16)
            wv_sb = consts.tile([128, 4, _HD], bf16)
            wo_sb = consts.tile([_HD, 4, 128], bf16)
            nc.scalar.dma_start(out=wq_sb, in_=wq_d[:, :, :])
            nc.scalar.dma_start(out=wk_sb, in_=wk_d[:, :, :])
            nc.scalar.dma_start(out=wv_sb, in_=wv_d[:, :, :])
            nc.scalar.dma_start(out=wo_sb, in_=wo_d[:, :, :])
            if bq_nonzero:
                bq_sb = consts.tile([_HD, 1], f32)
                nc.scalar.dma_start(out=bq_sb, in_=bq_d[:, :])
            wdummy = consts.tile([128, 128], bf16)
            nc.vector.memset(wdummy, 0.125)

            # persistent per-head tensors
            q2_sb = persist.tile([_HD, T], bf16)         # Q^T (parts 0-63)
            k2_sb = persist.tile([_HD, NK, 128], bf16)   # K^T granules
            v_all = persist.tile([128, NK, _HD + 1], bf16)  # V rows + ones
            o2_sb = persist.tile([_HD + 1, NQB, 512], bf16)  # AV out + sums

            # granule-tail init: zero K^T tails, zero V tails, set ones col
            for s in range(nseg):
                g0 = slot_base[s]
                nfull = seg_len[s] // 128
                rem = seg_len[s] - nfull * 128
                if nfull:
                    nc.gpsimd.memset(v_all[:, g0:g0 + nfull, _HD:_HD + 1], 1.0)
                if rem:
                    g = g0 + nfull
                    nc.vector.memset(k2_sb[:, g, rem:128], 0.0)
                    nc.gpsimd.memset(v_all[:, g, :], 0.0)
                    nc.gpsimd.memset(v_all[0:rem, g, _HD:_HD + 1], 1.0)

            # -------- input DMAs: scalar queue, priority order, throttled
            # (emitted just-in-time from the main loop so exp instructions
            # behind them on the queue never wait on slot semaphores)
            xq_tiles, xkp_tiles, xk_tiles = {}, {}, {}

            def emit_in(bi):
                if bi >= NQB or bi in xq_tiles:
                    return
                s, qb0, W, goff = blocks[bi]
                xq_t = xin.tile([128, 4, 512], bf16, tag="xq", name="xq",
                                bufs=2)
                xkp_t = xin.tile([128, 4, 512], f8e4, tag="xkp", name="xkp",
                                 bufs=2)
                xk_t = xin.tile([128, 4, 512], bf16, tag="xk", name="xk",
                                bufs=4)
                nc.scalar.dma_start(out=xq_t[:, :, 0:W],
                                    in_=xqp_d[:, :, goff:goff + W])
                nc.scalar.dma_start(out=xkp_t[:, :, 0:W],
                                    in_=xkp_d[:, :, goff:goff + W])
                nc.scalar.dma_start(out=xk_t[:, :, 0:W],
                                    in_=xk_d[:, :, goff:goff + W])
                xq_tiles[bi], xkp_tiles[bi], xk_tiles[bi] = xq_t, xkp_t, xk_t

            emit_in(0)
            emit_in(1)

            # -------- warmup: lift the PE clock gate during the DMA head --
            warm = scp.tile([128, 4, 512], f32, tag="sc", name="warm")
            for _ in range(30):
                nc.tensor.matmul(warm[:, 0, 0:128], lhsT=wdummy, rhs=wdummy,
                                 start=True, stop=True)

            # -------- emission helpers ------------------------------------
            pj_ps = {}

            def qk_mm(bi):
                s, qb0, W, goff = blocks[bi]
                ps = pjp.tile([128, 512], f32, tag="pj", name="qkps")
                for c in range(4):
                    nc.tensor.matmul(ps[0:_HD, 0:W], lhsT=wq_sb[:, c],
                                     rhs=xq_tiles[bi][:, c, 0:W],
                                     start=(c == 0), stop=(c == 3))
                    nc.tensor.matmul(ps[_HD:128, 0:W], lhsT=wk_sb[:, c],
                                     rhs=xkp_tiles[bi][:, c, 0:W],
                                     start=(c == 0), stop=(c == 3))
                pj_ps[bi] = ps

            def qk_cast(bi):
                s, qb0, W, goff = blocks[bi]
                ps = pj_ps.pop(bi)
                nc.vector.tensor_copy(out=q2_sb[:, goff:goff + W],
                                      in_=ps[0:_HD, 0:W])
                if bq_nonzero:
                    nc.vector.tensor_scalar_add(
                        q2_sb[:, goff:goff + W], q2_sb[:, goff:goff + W],
                        bq_sb[:, 0:1])
                l0 = slot_base[s] + qb0 // 128
                nbf, rem = W // 128, W % 128
                if nbf:
                    nc.vector.tensor_copy(
                        out=k2_sb[:, l0:l0 + nbf, :],
                        in_=ps[_HD:128, 0:nbf * 128].rearrange(
                            "p (b t) -> p b t", b=nbf, t=128))
                if rem:
                    nc.vector.tensor_copy(
                        out=k2_sb[:, l0 + nbf, 0:rem],
                        in_=ps[_HD:128, nbf * 128:W])

            def v_mm(bi):
                s, qb0, W, goff = blocks[bi]
                ps = pjp.tile([128, 512], f32, tag="pj", name="vtps")
                for c in range(4):
                    nc.tensor.matmul(ps[0:_HD, 0:W], lhsT=wv_sb[:, c],
                                     rhs=xk_tiles[bi][:, c, 0:W],
                                     start=(c == 0), stop=(c == 3))
                pj_ps[bi] = ps

            def v_tail(bi):
                s, qb0, W, goff = blocks[bi]
                ps = pj_ps.pop(bi)
                nb = (W + 127) // 128
                nbf, rem = W // 128, W % 128
                vt = sbw.tile([_HD, 512], bf16, tag="vt", name="vt")
                nc.vector.tensor_copy(out=vt[:, 0:nb * 128],
                                      in_=ps[0:_HD, 0:nb * 128])
                stag = sbw.tile([128, 4, _HD], bf16, tag="stag", name="stag")
                nc.sync.dma_start_transpose(stag[:, 0:nb, :],
                                            vt[:, 0:nb * 128])
                l0 = slot_base[s] + qb0 // 128
                if nbf:
                    nc.gpsimd.tensor_copy(
                        out=v_all[:, l0:l0 + nbf, 0:_HD],
                        in_=stag[:, 0:nbf, :])
                if rem:
                    nc.gpsimd.tensor_copy(
                        out=v_all[0:rem, l0 + nbf, 0:_HD],
                        in_=stag[0:rem, nbf, :])

            # -------- filler machinery ------------------------------------
            # dq: AV/outproj/V work in FIFO order -- popped between score
            #     batches to keep the PE busy while exp (the pacer) runs.
            # dq_proj: next segment's QK projections -- popped only when dq
            #     is empty, force-drained right before that segment's scores.
            dq = deque()       # (cost_ns, fn)
            dq_proj = deque()
            debt = [0.0]

            def pop_fillers():
                while debt[0] > 0:
                    if dq:
                        cost, fn = dq.popleft()
                    elif dq_proj:
                        cost, fn = dq_proj.popleft()
                    else:
                        break
                    fn()
                    debt[0] -= cost

            def drain_proj():
                while dq_proj:
                    _, fn = dq_proj.popleft()
                    fn()

            def drain_all():
                drain_proj()
                while dq:
                    _, fn = dq.popleft()
                    fn()
                debt[0] = 0.0

            def enq_qk(s):
                for bi, (s2, qb0, W, goff) in enumerate(blocks):
                    if s2 != s:
                        continue
                    dq_proj.append((W * 1.7, lambda b=bi: qk_mm(b)))
                    dq_proj.append((0.0, lambda b=bi: qk_cast(b)))

            def enq_v(s):
                for bi, (s2, qb0, W, goff) in enumerate(blocks):
                    if s2 != s:
                        continue
                    dq.append((W * 0.85, lambda b=bi: v_mm(b)))
                    dq.append((0.0, lambda b=bi: v_tail(b)))

            ex_tiles = {}

            def emit_scores(j):
                s, qb0, W, goff = blocks[j]
                nkt = nkt_s[s]
                sb = slot_base[s]
                ex = expp.tile([128, max_nkt, 512], bf16, tag="ex", name="ex")
                ex_tiles[j] = ex
                lk = 0
                while lk < nkt:
                    G = min(4, nkt - lk)
                    sc = scp.tile([128, 4, 512], f32, tag="sc", name="sc")
                    for g in range(G):
                        nc.tensor.matmul(
                            sc[:, g, 0:W], lhsT=k2_sb[:, sb + lk + g, :],
                            rhs=q2_sb[:, goff:goff + W],
                            start=True, stop=True)
                    nc.scalar.activation(
                        out=ex[:, lk:lk + G, 0:W], in_=sc[:, 0:G, 0:W],
                        func=mybir.ActivationFunctionType.Exp, scale=0.125)
                    lk += G
                    debt[0] += G * W * 0.66
                    pop_fillers()

            def enq_av(j):
                s, qb0, W, goff = blocks[j]
                nkt = nkt_s[s]
                sb = slot_base[s]
                st = {}

                def av_l(l):
                    if l == 0:
                        st["o"] = avp.tile([_HD + 1, 512], f32, tag="av",
                                           name="ops")
                    nc.tensor.matmul(
                        st["o"][:, 0:W], lhsT=v_all[:, sb + l, :],
                        rhs=ex_tiles[j][:, l, 0:W],
                        start=(l == 0), stop=(l == nkt - 1))

                def o2cast():
                    nc.vector.tensor_copy(out=o2_sb[:, j, 0:W],
                                          in_=st["o"][:, 0:W])
                    ex_tiles.pop(j)

                for l in range(nkt):
                    dq.append((W * 0.43, lambda l=l: av_l(l)))
                dq.append((0.0, o2cast))

            def enq_outproj(j):
                s, qb0, W, goff = blocks[j]
                st = {}

                def op_c(c):
                    if c == 0:
                        st["f"] = fins.tile([128, 4, 512], bf16, tag="fin",
                                            name="fin")
                    f_ps = fip.tile([128, 512], f32, tag="fi", name="fps")
                    nc.tensor.matmul(f_ps[:, 0:W], lhsT=wo_sb[:, c, :],
                                     rhs=o2_sb[0:_HD, j, 0:W],
                                     start=True, stop=True)
                    nc.vector.tensor_copy(out=st["f"][:, c, 0:W],
                                          in_=f_ps[:, 0:W])

                def out_dma():
                    nc.gpsimd.dma_start(out=outT_d[:, :, goff:goff + W],
                                        in_=st["f"][:, :, 0:W])

                for c in range(4):
                    dq.append((W * 0.43, lambda c=c: op_c(c)))
                dq.append((0.0, out_dma))

            # -------- main pipeline ---------------------------------------
            first_seg = blocks[0][0]
            for bi, (s2, qb0, W, goff) in enumerate(blocks):
                if s2 == first_seg:
                    qk_mm(bi)
                    qk_cast(bi)

            seg_of = [b[0] for b in blocks]
            for j, (s, qb0, W, goff) in enumerate(blocks):
                emit_in(j + 2)
                if qb0 == 0:
                    if j > 0:
                        drain_proj()  # finish this segment's QK projections
                    enq_v(s)
                emit_scores(j)
                enq_av(j)
                if j > 0:
                    enq_outproj(j - 1)
                if qb0 == 0:
                    nxt = None
                    for jj in range(j + 1, NQB):
                        if seg_of[jj] != s:
                            nxt = seg_of[jj]
                            break
                    if nxt is not None:
                        enq_qk(nxt)
            drain_all()
            enq_outproj(NQB - 1)
            drain_all()
            nc.gpsimd.dma_start(out=sums_d[:, :, :],
                                in_=o2_sb[_HD:_HD + 1, :, :])

    nc.compile()
    return nc


def _prep_T(x, T, dtype):
    """[T, D] f32 -> [128, 4, T]: element (p, c, t) = x[t, c*128+p]."""
    xt = x.T.reshape(4, 128, T)  # [c, p, t]
    return np.ascontiguousarray(xt.transpose(1, 0, 2)).astype(dtype)


def kernel(x_query, x_keyval, pos, channels, Wq, bq, Wk, bk, Wv, bv, Wo, bo,
           _trace=False, _trace_cores=None):
    _ensure_ntff_hook()
    import concourse.bass_utils as bu

    bu.upload_artifacts = lambda tmpdir: tmpdir  # no S3 egress from here

    x_query = np.asarray(x_query, dtype=np.float32)
    x_keyval = np.asarray(x_keyval, dtype=np.float32)
    pos = np.asarray(pos, dtype=np.float32)
    channels = np.asarray(channels)
    Wq, bq = np.asarray(Wq, np.float32), np.asarray(bq, np.float32)
    Wk, bk = np.asarray(Wk, np.float32), np.asarray(bk, np.float32)
    Wv, bv = np.asarray(Wv, np.float32), np.asarray(bv, np.float32)
    Wo, bo = np.asarray(Wo, np.float32), np.asarray(bo, np.float32)

    C, S, D = x_query.shape
    seg_len, toff, T = _segments(channels)
    assert sum(seg_len) == C * S, "channels inconsistent with batch dim"
    P = _plan(seg_len, toff, T)

    bq_nonzero = bool(np.any(bq))
    seg_key = (tuple(seg_len), tuple(toff), T)
    cache_key = (seg_key, bq_nonzero)
    if cache_key not in _prog_cache:
        _prog_cache[cache_key] = _build_program(seg_key, bq_nonzero)
    nc = _prog_cache[cache_key]

    bf = ml_dtypes.bfloat16
    f8 = ml_dtypes.float8_e4m3fn
    xq_flat = x_query.reshape(-1, D)
    xk_flat = x_keyval.reshape(-1, D)
    p_flat = pos.reshape(-1, D)
    xqp_b = _prep_T(xq_flat + p_flat, T, bf)
    xkp_b = _prep_T(xk_flat + p_flat, T, f8)
    xk_b = _prep_T(xk_flat, T, bf)

    def wchunk(w):  # [512, 64] -> [128, 4, 64]
        return np.ascontiguousarray(
            w.reshape(4, 128, _HD).transpose(1, 0, 2)).astype(bf)

    in_maps = []
    for h in range(_NCORES):
        sl = slice(h * _HD, (h + 1) * _HD)
        m = {
            "xqp": xqp_b,
            "xkp": xkp_b,
            "xk": xk_b,
            "wq": wchunk(Wq[:, sl]),
            "wk": wchunk(Wk[:, sl]),
            "wv": wchunk(Wv[:, sl]),
            "wo": np.ascontiguousarray(
                Wo[sl, :].reshape(_HD, 4, 128)).astype(bf),
        }
        if bq_nonzero:
            m["bq"] = np.ascontiguousarray(
                bq[sl].reshape(_HD, 1)).astype(np.float32)
        in_maps.append(m)

    from concourse.bass_utils import run_bass_kernel_spmd

    kwargs = {}
    if _trace:
        kwargs["trace"] = True
        if _trace_cores is not None:
            kwargs["trace_cores"] = _trace_cores
    res = run_bass_kernel_spmd(nc, in_maps, list(range(_NCORES)), **kwargs)

    # host gather: per-head softmax normalization + sum + bias folds
    blocks = P["blocks"]
    acc = np.zeros((D, T), dtype=np.float64)
    with np.errstate(divide="ignore", invalid="ignore"):
        for h in range(_NCORES):
            outT = np.asarray(res.results[h]["outT"], dtype=np.float64)
            sums_r = np.asarray(res.results[h]["sums"], dtype=np.float64)
            sums = np.empty(T, dtype=np.float64)
            for j, (s, qb0, W, goff) in enumerate(blocks):
                sums[goff:goff + W] = sums_r[0, j, 0:W]
            sums = np.where(sums == 0.0, 1.0, sums)
            acc += outT.transpose(1, 0, 2).reshape(D, T) / sums[None, :]

    const = bv @ Wo + bo  # bias fold (exact; zero in the spec's fills)

    out = (acc.T + const[None, :]).astype(np.float32).reshape(C, S, D)

    if _trace:
        kernel._last_exec_time_ns = res.exec_time_ns
        kernel._last_trace = (
            res.instructions_and_trace[1] if res.instructions_and_trace else None
        )
    return out


# revision 14
# speedup vs baseline: 1.1045x; 1.0068x over previous
"""Bass/Trainium2 kernel for a ragged-sequence CrossAttentionBlock.

Math (per reference):
  T = 16*196 packed tokens, D=512, H=8 heads of HD=64.
  q = (xq + pos) @ Wq + bq ; k = (xk + pos) @ Wk + bk ; v = xk @ Wv + bv
  block-diagonal attention over segments of channels[i]*196 tokens
  out = softmax(q k^T / 8) v  -> concat heads -> @ Wo + bo

Sharding: one head per NeuronCore. Each core computes its head's Q/K/V
over all tokens, the per-segment attention, and its head's slice of the
output projection Wo[h*64:(h+1)*64, :]. Host sums per-head projections.

Bias folding (exact):
  bk: uniform per-query logit shift -> softmax-invariant, drops.
  bq: per-partition add on Q^T after projection (emitted only if nonzero).
  bv, bo: commute through softmax (rows sum to 1): host adds bv@Wo + bo.

v2 design notes (vs the padded baseline):
  - tight token axis: inputs/outputs are [128, 4, T] with NO inter-segment
    padding; projection blocks coincide with the 512-token query blocks of
    each segment, so K/V granules align to segment starts.
  - all input DMAs ride ONE queue (sync) in priority order, throttled by
    per-tag pool bufs, so the first block lands in ~4us instead of ~23us.
  - K^T tiles all live in partitions 0-63 (copied from the QK PSUM rows
    64-127 with a partition shift), so Q^T needs no duplicate.
  - softmax sums ride the AV PSUM as row 64 and are cast into row 64 of
    the o2 staging tensor (bf16) -- no separate sums copy.
  - a software pipeline at query-block granularity: while block j's scores
    stream and exp (the pacer, on the scalar engine) runs, the PE executes
    "filler" work -- AV of block j-1, outproj of block j-2, projections of
    the next segment -- popped from a deque sized by an ACT-vs-PE debt
    estimate. PSUM: proj(1) + scores(4) + av(1) + outproj(2) = 8 banks.
  - dtypes: everything bf16 except xkp (fp8e4m3, as the error budget
    allows exactly one fp8 quantization under the 2e-2 gate).
"""

import sys
import types
from collections import deque

import numpy as np
import ml_dtypes

_D = 512
_HD = 64
_H = 8
_S = 196
_NCORES = 8

_prog_cache = {}


def _ensure_ntff_hook():
    """Register the NTFF profile hook that the agent image's antenv lacks."""
    if "antenv.axon_hooks" in sys.modules:
        return
    try:
        from trn_agent_boot.trn_boot import _ntff_profile_via_ctypes

        hook = _ntff_profile_via_ctypes("/opt/axon/libaxon_pjrt.so")
        mod = types.ModuleType("antenv.axon_hooks")
        mod.get_axon_ntff_profile_hook = lambda: hook
        sys.modules["antenv.axon_hooks"] = mod
    except Exception:
        pass


def _segments(channels):
    seg_len = [int(c) * _S for c in np.asarray(channels).tolist() if int(c) > 0]
    toff = []
    o = 0
    for l in seg_len:
        toff.append(o)
        o += l
    return seg_len, toff, o  # lengths, tight offsets, T


def _plan(seg_len, toff, T):
    nseg = len(seg_len)
    nkt_s = [(l + 127) // 128 for l in seg_len]
    slot_base = []
    b = 0
    for s in range(nseg):
        slot_base.append(b)
        b += nkt_s[s]
    NK = b
    p_order = sorted(range(nseg), key=lambda s: (seg_len[s], s))
    blocks = []  # (s, qb0, W, goff)
    for s in p_order:
        for qb0 in range(0, seg_len[s], 512):
            W = min(512, seg_len[s] - qb0)
            blocks.append((s, qb0, W, toff[s] + qb0))
    return dict(nseg=nseg, nkt_s=nkt_s, slot_base=slot_base, NK=NK,
                p_order=p_order, blocks=blocks, NQB=len(blocks),
                max_nkt=max(nkt_s))


def _build_program(seg_key, bq_nonzero):
    import concourse.bacc as bacc
    import concourse.tile as tile
    from concourse import mybir

    seg_len, toff, T = seg_key
    seg_len, toff = list(seg_len), list(toff)
    P = _plan(seg_len, toff, T)
    nseg, nkt_s, slot_base = P["nseg"], P["nkt_s"], P["slot_base"]
    NK, blocks, NQB, max_nkt = P["NK"], P["blocks"], P["NQB"], P["max_nkt"]

    f32 = mybir.dt.float32
    bf16 = mybir.dt.bfloat16
    f8e4 = mybir.dt.float8e4

    nc = bacc.Bacc("TRN2", target_bir_lowering=False, debug=False,
                   num_devices=_NCORES)

    xqp_d = nc.dram_tensor("xqp", [128, 4, T], bf16, kind="ExternalInput")
    xkp_d = nc.dram_tensor("xkp", [128, 4, T], f8e4, kind="ExternalInput")
    xk_d = nc.dram_tensor("xk", [128, 4, T], bf16, kind="ExternalInput")
    wq_d = nc.dram_tensor("wq", [128, 4, _HD], bf16, kind="ExternalInput")
    wk_d = nc.dram_tensor("wk", [128, 4, _HD], bf16, kind="ExternalInput")
    wv_d = nc.dram_tensor("wv", [128, 4, _HD], bf16, kind="ExternalInput")
    wo_d = nc.dram_tensor("wo", [_HD, 4, 128], bf16, kind="ExternalInput")
    if bq_nonzero:
        bq_d = nc.dram_tensor("bq", [_HD, 1], f32, kind="ExternalInput")
    outT_d = nc.dram_tensor("outT", [128, 4, T], bf16, kind="ExternalOutput")
    sums_d = nc.dram_tensor("sums", [1, NQB, 512], bf16,
                            kind="ExternalOutput")

    with tile.TileContext(nc) as tc:
        with (
            tc.tile_pool(name="consts", bufs=1) as consts,
            tc.tile_pool(name="persist", bufs=1) as persist,
            tc.tile_pool(name="xin", bufs=2) as xin,
            tc.tile_pool(name="sbw", bufs=2) as sbw,
            tc.tile_pool(name="expp", bufs=3) as expp,
            tc.tile_pool(name="fins", bufs=2) as fins,
            tc.tile_pool(name="pjp", bufs=1, space="PSUM") as pjp,
            tc.tile_pool(name="scp", bufs=1, space="PSUM") as scp,
            tc.tile_pool(name="avp", bufs=1, space="PSUM") as avp,
            tc.tile_pool(name="fip", bufs=2, space="PSUM") as fip,
        ):
            wq_sb = consts.tile([128, 4, _HD], bf16)
            wk_sb = consts.tile([128, 4, _HD], bf16)
            wv_sb = consts.tile([128, 4, _HD], bf16)
            wo_sb = consts.tile([_HD, 4, 128], bf16)
            nc.scalar.dma_start(out=wq_sb, in_=wq_d[:, :, :])
            nc.scalar.dma_start(out=wk_sb, in_=wk_d[:, :, :])
            nc.scalar.dma_start(out=wv_sb, in_=wv_d[:, :, :])
            nc.scalar.dma_start(out=wo_sb, in_=wo_d[:, :, :])
            if bq_nonzero:
                bq_sb = consts.tile([_HD, 1], f32)
                nc.scalar.dma_start(out=bq_sb, in_=bq_d[:, :])
            wdummy = consts.tile([128, 128], bf16)
            nc.vector.memset(wdummy, 0.125)

            # persistent per-head tensors
            q2_sb = persist.tile([_HD, T], bf16)         # Q^T (parts 0-63)
            k2_sb = persist.tile([_HD, NK, 128], bf16)   # K^T granules
            v_all = persist.tile([128, NK, _HD + 1], bf16)  # V rows + ones
            o2_sb = persist.tile([_HD + 1, NQB, 512], bf16)  # AV out + sums

            # granule-tail init: zero K^T tails, zero V tails, set ones col
            for s in range(nseg):
                g0 = slot_base[s]
                nfull = seg_len[s] // 128
                rem = seg_len[s] - nfull * 128
                if nfull:
                    nc.gpsimd.memset(v_all[:, g0:g0 + nfull, _HD:_HD + 1], 1.0)
                if rem:
                    g = g0 + nfull
                    nc.vector.memset(k2_sb[:, g, rem:128], 0.0)
                    nc.gpsimd.memset(v_all[:, g, :], 0.0)
                    nc.gpsimd.memset(v_all[0:rem, g, _HD:_HD + 1], 1.0)

            # -------- input DMAs: scalar queue, priority order, throttled
            # (emitted just-in-time from the main loop so exp instructions
            # behind them on the queue never wait on slot semaphores)
            xq_tiles, xkp_tiles, xk_tiles = {}, {}, {}

            def emit_in(bi):
                if bi >= NQB or bi in xq_tiles:
                    return
                s, qb0, W, goff = blocks[bi]
                xq_t = xin.tile([128, 4, 512], bf16, tag="xq", name="xq",
                                bufs=2)
                xkp_t = xin.tile([128, 4, 512], f8e4, tag="xkp", name="xkp",
                                 bufs=2)
                xk_t = xin.tile([128, 4, 512], bf16, tag="xk", name="xk",
                                bufs=4)
                nc.scalar.dma_start(out=xq_t[:, :, 0:W],
                                    in_=xqp_d[:, :, goff:goff + W])
                nc.scalar.dma_start(out=xkp_t[:, :, 0:W],
                                    in_=xkp_d[:, :, goff:goff + W])
                nc.scalar.dma_start(out=xk_t[:, :, 0:W],
                                    in_=xk_d[:, :, goff:goff + W])
                xq_tiles[bi], xkp_tiles[bi], xk_tiles[bi] = xq_t, xkp_t, xk_t

            emit_in(0)
            emit_in(1)

            # -------- warmup: lift the PE clock gate during the DMA head --
            warm = scp.tile([128, 4, 512], f32, tag="sc", name="warm")
            for _ in range(30):
                nc.tensor.matmul(warm[:, 0, 0:128], lhsT=wdummy, rhs=wdummy,
                                 start=True, stop=True)

            # -------- emission helpers ------------------------------------
            pj_ps = {}

            def qk_mm(bi):
                s, qb0, W, goff = blocks[bi]
                ps = pjp.tile([128, 512], f32, tag="pj", name="qkps")
                for c in range(4):
                    nc.tensor.matmul(ps[0:_HD, 0:W], lhsT=wq_sb[:, c],
                                     rhs=xq_tiles[bi][:, c, 0:W],
                                     start=(c == 0), stop=(c == 3))
                    nc.tensor.matmul(ps[_HD:128, 0:W], lhsT=wk_sb[:, c],
                                     rhs=xkp_tiles[bi][:, c, 0:W],
                                     start=(c == 0), stop=(c == 3))
                pj_ps[bi] = ps

            def qk_cast(bi):
                s, qb0, W, goff = blocks[bi]
                ps = pj_ps.pop(bi)
                nc.vector.tensor_copy(out=q2_sb[:, goff:goff + W],
                                      in_=ps[0:_HD, 0:W])
                if bq_nonzero:
                    nc.vector.tensor_scalar_add(
                        q2_sb[:, goff:goff + W], q2_sb[:, goff:goff + W],
                        bq_sb[:, 0:1])
                l0 = slot_base[s] + qb0 // 128
                nbf, rem = W // 128, W % 128
                if nbf:
                    nc.vector.tensor_copy(
                        out=k2_sb[:, l0:l0 + nbf, :],
                        in_=ps[_HD:128, 0:nbf * 128].rearrange(
                            "p (b t) -> p b t", b=nbf, t=128))
                if rem:
                    nc.vector.tensor_copy(
                        out=k2_sb[:, l0 + nbf, 0:rem],
                        in_=ps[_HD:128, nbf * 128:W])

            def v_mm(bi):
                s, qb0, W, goff = blocks[bi]
                ps = pjp.tile([128, 512], f32, tag="pj", name="vtps")
                for c in range(4):
                    nc.tensor.matmul(ps[0:_HD, 0:W], lhsT=wv_sb[:, c],
                                     rhs=xk_tiles[bi][:, c, 0:W],
                                     start=(c == 0), stop=(c == 3))
                pj_ps[bi] = ps

            def v_tail(bi):
                s, qb0, W, goff = blocks[bi]
                ps = pj_ps.pop(bi)
                nb = (W + 127) // 128
                nbf, rem = W // 128, W % 128
                vt = sbw.tile([_HD, 512], bf16, tag="vt", name="vt", bufs=3)
                nc.vector.tensor_copy(out=vt[:, 0:nb * 128],
                                      in_=ps[0:_HD, 0:nb * 128])
                stag = sbw.tile([128, 4, _HD], bf16, tag="stag", name="stag",
                                bufs=4)
                nc.sync.dma_start_transpose(stag[:, 0:nb, :],
                                            vt[:, 0:nb * 128])
                l0 = slot_base[s] + qb0 // 128
                if nbf:
                    nc.gpsimd.tensor_copy(
                        out=v_all[:, l0:l0 + nbf, 0:_HD],
                        in_=stag[:, 0:nbf, :])
                if rem:
                    nc.gpsimd.tensor_copy(
                        out=v_all[0:rem, l0 + nbf, 0:_HD],
                        in_=stag[0:rem, nbf, :])

            # -------- filler machinery ------------------------------------
            # dq: AV/outproj/V work in FIFO order -- popped between score
            #     batches to keep the PE busy while exp (the pacer) runs.
            # dq_proj: next segment's QK projections -- popped only when dq
            #     is empty, force-drained right before that segment's scores.
            dq = deque()       # (cost_ns, fn)
            dq_proj = deque()
            debt = [0.0]

            def pop_fillers():
                while debt[0] > 0:
                    if dq:
                        cost, fn = dq.popleft()
                    elif dq_proj:
                        cost, fn = dq_proj.popleft()
                    else:
                        break
                    fn()
                    debt[0] -= cost

            def drain_proj():
                while dq_proj:
                    _, fn = dq_proj.popleft()
                    fn()

            def drain_all():
                drain_proj()
                while dq:
                    _, fn = dq.popleft()
                    fn()
                debt[0] = 0.0

            def enq_qk(s):
                for bi, (s2, qb0, W, goff) in enumerate(blocks):
                    if s2 != s:
                        continue
                    dq_proj.append((W * 1.7, lambda b=bi: qk_mm(b)))
                    dq_proj.append((0.0, lambda b=bi: qk_cast(b)))

            def enq_v(s):
                # rides dq_proj: V work is emitted a segment ahead so the
                # long vt->transpose->pool chain lands before AV needs it
                for bi, (s2, qb0, W, goff) in enumerate(blocks):
                    if s2 != s:
                        continue
                    dq_proj.append((W * 0.85, lambda b=bi: v_mm(b)))
                    dq_proj.append((0.0, lambda b=bi: v_tail(b)))

            ex_tiles = {}

            def emit_scores(j):
                s, qb0, W, goff = blocks[j]
                nkt = nkt_s[s]
                sb = slot_base[s]
                ex = expp.tile([128, max_nkt, 512], bf16, tag="ex", name="ex")
                ex_tiles[j] = ex
                lk = 0
                while lk < nkt:
                    G = min(4, nkt - lk)
                    sc = scp.tile([128, 4, 512], f32, tag="sc", name="sc")
                    for g in range(G):
                        nc.tensor.matmul(
                            sc[:, g, 0:W], lhsT=k2_sb[:, sb + lk + g, :],
                            rhs=q2_sb[:, goff:goff + W],
                            start=True, stop=True)
                    nc.scalar.activation(
                        out=ex[:, lk:lk + G, 0:W], in_=sc[:, 0:G, 0:W],
                        func=mybir.ActivationFunctionType.Exp, scale=0.125)
                    lk += G
                    debt[0] += G * W * 0.66
                    pop_fillers()

            def enq_av(j):
                s, qb0, W, goff = blocks[j]
                nkt = nkt_s[s]
                sb = slot_base[s]
                st = {}

                def av_l(l):
                    if l == 0:
                        st["o"] = avp.tile([_HD + 1, 512], f32, tag="av",
                                           name="ops")
                    nc.tensor.matmul(
                        st["o"][:, 0:W], lhsT=v_all[:, sb + l, :],
                        rhs=ex_tiles[j][:, l, 0:W],
                        start=(l == 0), stop=(l == nkt - 1))

                def o2cast():
                    nc.vector.tensor_copy(out=o2_sb[:, j, 0:W],
                                          in_=st["o"][:, 0:W])
                    ex_tiles.pop(j)

                for l in range(nkt):
                    dq.append((W * 0.43, lambda l=l: av_l(l)))
                dq.append((0.0, o2cast))

            def enq_outproj(j):
                s, qb0, W, goff = blocks[j]
                st = {}

                def op_c(c):
                    if c == 0:
                        st["f"] = fins.tile([128, 4, 512], bf16, tag="fin",
                                            name="fin")
                    f_ps = fip.tile([128, 512], f32, tag="fi", name="fps")
                    nc.tensor.matmul(f_ps[:, 0:W], lhsT=wo_sb[:, c, :],
                                     rhs=o2_sb[0:_HD, j, 0:W],
                                     start=True, stop=True)
                    nc.vector.tensor_copy(out=st["f"][:, c, 0:W],
                                          in_=f_ps[:, 0:W])

                def out_dma():
                    nc.scalar.dma_start(out=outT_d[:, :, goff:goff + W],
                                        in_=st["f"][:, :, 0:W])

                for c in range(4):
                    dq.append((W * 0.43, lambda c=c: op_c(c)))
                dq.append((0.0, out_dma))

            # -------- main pipeline ---------------------------------------
            first_seg = blocks[0][0]
            for bi, (s2, qb0, W, goff) in enumerate(blocks):
                if s2 == first_seg:
                    qk_mm(bi)
                    qk_cast(bi)
                    v_mm(bi)
                    v_tail(bi)

            seg_of = [b[0] for b in blocks]
            for j, (s, qb0, W, goff) in enumerate(blocks):
                emit_in(j + 2)
                if qb0 == 0 and j > 0:
                    drain_proj()  # finish this segment's QK + V projections
                emit_scores(j)
                enq_av(j)
                if j > 0:
                    enq_outproj(j - 1)
                if qb0 == 0:
                    nxt = None
                    for jj in range(j + 1, NQB):
                        if seg_of[jj] != s:
                            nxt = seg_of[jj]
                            break
                    if nxt is not None:
                        enq_qk(nxt)
                        enq_v(nxt)
            drain_all()
            enq_outproj(NQB - 1)
            drain_all()
            nc.scalar.dma_start(out=sums_d[:, :, :],
                                in_=o2_sb[_HD:_HD + 1, :, :])

    nc.compile()
    return nc


def _prep_T(x, T, dtype):
    """[T, D] f32 -> [128, 4, T]: element (p, c, t) = x[t, c*128+p]."""
    xt = x.T.reshape(4, 128, T)  # [c, p, t]
    return np.ascontiguousarray(xt.transpose(1, 0, 2)).astype(dtype)


def kernel(x_query, x_keyval, pos, channels, Wq, bq, Wk, bk, Wv, bv, Wo, bo,
           _trace=False, _trace_cores=None):
    _ensure_ntff_hook()
    import concourse.bass_utils as bu

    bu.upload_artifacts = lambda tmpdir: tmpdir  # no S3 egress from here

    x_query = np.asarray(x_query, dtype=np.float32)
    x_keyval = np.asarray(x_keyval, dtype=np.float32)
    pos = np.asarray(pos, dtype=np.float32)
    channels = np.asarray(channels)
    Wq, bq = np.asarray(Wq, np.float32), np.asarray(bq, np.float32)
    Wk, bk = np.asarray(Wk, np.float32), np.asarray(bk, np.float32)
    Wv, bv = np.asarray(Wv, np.float32), np.asarray(bv, np.float32)
    Wo, bo = np.asarray(Wo, np.float32), np.asarray(bo, np.float32)

    C, S, D = x_query.shape
    seg_len, toff, T = _segments(channels)
    assert sum(seg_len) == C * S, "channels inconsistent with batch dim"
    P = _plan(seg_len, toff, T)

    bq_nonzero = bool(np.any(bq))
    seg_key = (tuple(seg_len), tuple(toff), T)
    cache_key = (seg_key, bq_nonzero)
    if cache_key not in _prog_cache:
        _prog_cache[cache_key] = _build_program(seg_key, bq_nonzero)
    nc = _prog_cache[cache_key]

    bf = ml_dtypes.bfloat16
    f8 = ml_dtypes.float8_e4m3fn
    xq_flat = x_query.reshape(-1, D)
    xk_flat = x_keyval.reshape(-1, D)
    p_flat = pos.reshape(-1, D)
    xqp_b = _prep_T(xq_flat + p_flat, T, bf)
    xkp_b = _prep_T(xk_flat + p_flat, T, f8)
    xk_b = _prep_T(xk_flat, T, bf)

    def wchunk(w):  # [512, 64] -> [128, 4, 64]
        return np.ascontiguousarray(
            w.reshape(4, 128, _HD).transpose(1, 0, 2)).astype(bf)

    in_maps = []
    for h in range(_NCORES):
        sl = slice(h * _HD, (h + 1) * _HD)
        m = {
            "xqp": xqp_b,
            "xkp": xkp_b,
            "xk": xk_b,
            "wq": wchunk(Wq[:, sl]),
            "wk": wchunk(Wk[:, sl]),
            "wv": wchunk(Wv[:, sl]),
            "wo": np.ascontiguousarray(
                Wo[sl, :].reshape(_HD, 4, 128)).astype(bf),
        }
        if bq_nonzero:
            m["bq"] = np.ascontiguousarray(
                bq[sl].reshape(_HD, 1)).astype(np.float32)
        in_maps.append(m)

    from concourse.bass_utils import run_bass_kernel_spmd

    kwargs = {}
    if _trace:
        kwargs["trace"] = True
        if _trace_cores is not None:
            kwargs["trace_cores"] = _trace_cores
    res = run_bass_kernel_spmd(nc, in_maps, list(range(_NCORES)), **kwargs)

    # host gather: per-head softmax normalization + sum + bias folds
    blocks = P["blocks"]
    acc = np.zeros((D, T), dtype=np.float64)
    with np.errstate(divide="ignore", invalid="ignore"):
        for h in range(_NCORES):
            outT = np.asarray(res.results[h]["outT"], dtype=np.float64)
            sums_r = np.asarray(res.results[h]["sums"], dtype=np.float64)
            sums = np.empty(T, dtype=np.float64)
            for j, (s, qb0, W, goff) in enumerate(blocks):
                sums[goff:goff + W] = sums_r[0, j, 0:W]
            sums = np.where(sums == 0.0, 1.0, sums)
            acc += outT.transpose(1, 0, 2).reshape(D, T) / sums[None, :]

    const = bv @ Wo + bo  # bias fold (exact; zero in the spec's fills)

    out = (acc.T + const[None, :]).astype(np.float32).reshape(C, S, D)

    if _trace:
        kernel._last_exec_time_ns = res.exec_time_ns
        kernel._last_trace = (
            res.instructions_and_trace[1] if res.instructions_and_trace else None
        )
    return out
